# revision 13
# baseline (speedup 1.0000x reference)
"""Trainium2 Bass kernel for CRF NLL loss (nn_CRF) — time-sharded, 8 cores.

Each core owns a 128-step time segment for ALL 512 batch rows, split into
TWO 64-slot sub-segments (A: slots 0-63, B: 64-127). Each sub-segment runs
a stacked fwd+bwd chain (fwd partitions 0-47, bwd 64-112) as ONE full-width
[112,512] state against a block-diagonal [112,112] stationary: 31 fused
steps + a mid-slab combine. The two sub-segment chains interleave on the
engines, hiding the per-step mm->mul round-trip latency that bound the
single-segment version.

Norm telescoping: pz_s = 1^T alpha_hat(end of s). Sub-segment A's fwd seed
comes from the W=2 warmup (crafted exact on core 0, logged+cancelled with
weight rwrow elsewhere); B's fwd seed warms up locally from slots 62-63 and
its norm mxwB is always cancelled (weight -1). Backward chains start exact:
binitA = exp(em[slot 63]), binitB = exp(em[slot 127]) (+end on core 7).
loss_core = sum_b [ln pzA + ln pzB + rwr*ln mxwA - ln mxwB] + FINC - sums.

All exp() is on HOST (slabs/stationaries pre-exponentiated; 31-step chains
need no renorm: state ~1e-8, z-products ~1e-18, inside bf16/f32 range).
Tag one-hots (ohx) and raw emissions (empe) arrive as fp8e4m3 and feed the
numerator pair-matmuls, which the scheduler hoists into the scan as PE
filler — keeping the PE continuously busy also ramps its DVFS p-state so
scan matmuls run at full clock.
"""
import os
import sys

import numpy as np
import ml_dtypes

for _p in ("/opt/trn_rl_repo", "/root/.axon_site/_ro/trn_rl_repo"):
    if os.path.isdir(_p) and _p not in sys.path:
        sys.path.insert(0, _p)

import concourse.bass as bass
import concourse.bacc as bacc
import concourse.mybir as mybir
import concourse.tile as tile

B, S, T = 512, 1024, 48
NCORES = 8
SEG = S // NCORES            # 128 time slots owned per core
SUB = 64                     # slots per sub-segment
W = 2                        # fwd warmup steps
NF = 31                      # fused fwd/bwd steps per sub-segment
CBIAS = 4.9375               # folded into the host-side exp of every slab
CHUNK = 8                    # emstack cols per chunk (8 chunks of 8)
FINC = float(B * 2 * 64 * CBIAS)  # 512 rows * 128 slabs * CBIAS
GW = SEG * T                 # 6144 cols per row-group wave
OHW = (SEG + 1) * T          # 6192: boundary block + 128 slot blocks

BF16 = mybir.dt.bfloat16
FP8 = mybir.dt.float8e4
F32 = mybir.dt.float32
AL = mybir.AluOpType
AX = mybir.AxisListType
AF = mybir.ActivationFunctionType

bf16np = ml_dtypes.bfloat16
fp8np = ml_dtypes.float8_e4m3fn


def _build_graph():
    nc = bacc.Bacc("TRN2", target_bir_lowering=False, debug=False)

    emstack = nc.dram_tensor("emstack", [112, 64 * B], BF16, kind="ExternalInput")
    wsl = nc.dram_tensor("wsl", [112, W * B], BF16, kind="ExternalInput")
    binit = nc.dram_tensor("binit", [T, 2 * B], BF16, kind="ExternalInput")
    wstat = nc.dram_tensor("wstat", [112, 112], BF16, kind="ExternalInput")
    stat0 = nc.dram_tensor("stat0", [112, 112], BF16, kind="ExternalInput")
    bigmd = nc.dram_tensor("bigmd", [112, 112], BF16, kind="ExternalInput")
    trN96d = nc.dram_tensor("trN96d", [96, 96], F32, kind="ExternalInput")
    id96d = nc.dram_tensor("id96d", [96, 96], BF16, kind="ExternalInput")
    empe = nc.dram_tensor("empe", [128, 4 * GW], FP8, kind="ExternalInput")
    ohx = nc.dram_tensor("ohx", [128, 4 * OHW], FP8, kind="ExternalInput")
    edgeoh = nc.dram_tensor("edgeoh", [128, 4 * T], BF16, kind="ExternalInput")
    edgevec = nc.dram_tensor("edgevec", [T, 1], F32, kind="ExternalInput")
    rwrow = nc.dram_tensor("rwrow", [1, B], F32, kind="ExternalInput")
    outd = nc.dram_tensor("out", [1, 1], F32, kind="ExternalOutput")

    with tile.TileContext(nc) as tc:
        _kern(tc, nc, emstack, wsl, binit, wstat, stat0, bigmd, trN96d,
              id96d, empe, ohx, edgeoh, edgevec, rwrow, outd)
    nc.compile()
    return nc


def _kern(tc, nc, emstack, wsl, binit, wstat, stat0, bigmd, trN96d, id96d,
          empe, ohx, edgeoh, edgevec, rwrow, outd):
    from contextlib import ExitStack
    ctx = ExitStack()
    const = ctx.enter_context(tc.tile_pool(name="const", bufs=1))
    statep = ctx.enter_context(tc.tile_pool(name="state", bufs=3))
    psp = ctx.enter_context(tc.tile_pool(name="psp", bufs=1, space="PSUM"))
    psn = ctx.enter_context(tc.tile_pool(name="psn", bufs=1, space="PSUM"))
    psr = ctx.enter_context(tc.tile_pool(name="psr", bufs=1, space="PSUM"))
    psb = ctx.enter_context(tc.tile_pool(name="psb", bufs=2, space="PSUM"))
    psx = ctx.enter_context(tc.tile_pool(name="psx", bufs=1, space="PSUM"))
    rawp = ctx.enter_context(tc.tile_pool(name="raw", bufs=8))
    ohp = ctx.enter_context(tc.tile_pool(name="ohp", bufs=4))
    emp = ctx.enter_context(tc.tile_pool(name="emp", bufs=4))
    smallp = ctx.enter_context(tc.tile_pool(name="small", bufs=1))

    # ---------- scan-critical DMAs first ----------
    wslr = const.tile([112, W * B], BF16)
    nc.gpsimd.dma_start(wslr[:], wsl[:, :])
    wstat112 = const.tile([112, 112], BF16)
    nc.gpsimd.dma_start(wstat112[:], wstat[:, :])
    stat0t = const.tile([112, 112], BF16)
    nc.gpsimd.dma_start(stat0t[:], stat0[:, :])
    bigm = const.tile([112, 112], BF16)
    nc.gpsimd.dma_start(bigm[:], bigmd[:, :])
    raws = [rawp.tile([112, CHUNK * B], BF16, tag="raw", name=f"raw{ci}")
            for ci in range(8)]
    nc.sync.dma_start(raws[0][:], emstack[:, 0:CHUNK * B])

    # stacked state tiles per sub-segment: bwd rows DMA'd pre-exp'd;
    # fwd rows written by the warmup's last step
    SS = []
    for sub in (0, 1):
        st_ = statep.tile([112, B], BF16, tag=f"state{sub}", name=f"state{sub}")
        nc.vector.memset(st_[32:64, :], 0.0)
        nc.gpsimd.dma_start(st_[64:112, :], binit[:, sub * B:(sub + 1) * B])
        SS.append(st_)

    trN96 = const.tile([96, 96], F32)
    nc.gpsimd.dma_start(trN96[:], trN96d[:, :])
    id96 = const.tile([96, 96], BF16)
    nc.gpsimd.dma_start(id96[:], id96d[:, :])
    rwr = const.tile([1, B], F32)
    nc.gpsimd.dma_start(rwr[:], rwrow[:, :])
    edgt = const.tile([128, 4 * T], BF16)
    nc.gpsimd.dma_start(edgt[:], edgeoh[:, :])
    edgv = const.tile([T, 1], F32)
    nc.gpsimd.dma_start(edgv[:], edgevec[:, :])

    # site tiles, all resident
    emt = [emp.tile([128, GW], FP8, tag="em", name=f"em{g}") for g in range(4)]
    ohxt = [ohp.tile([128, OHW], FP8, tag="oh", name=f"oh{g}")
            for g in range(4)]

    def dma_em(g):
        hw = GW // 2
        for q in (0, 1):
            nc.sync.dma_start(emt[g][:, q * hw:(q + 1) * hw],
                              empe[:, g * GW + q * hw:g * GW + (q + 1) * hw])

    def dma_oh(g):
        hw = OHW // 2
        for q in (0, 1):
            nc.sync.dma_start(ohxt[g][:, q * hw:(q + 1) * hw],
                              ohx[:, g * OHW + q * hw:g * OHW + (q + 1) * hw])

    def dma_raw(ci):
        nc.sync.dma_start(raws[ci][:],
                          emstack[:, ci * CHUNK * B:(ci + 1) * CHUNK * B])

    # interleave raw chunks with site waves: filler available from scan start
    dma_oh(0); dma_em(0)
    dma_raw(1); dma_raw(2)
    dma_oh(1); dma_em(1)
    dma_raw(3); dma_raw(4)
    dma_oh(2); dma_em(2)
    dma_raw(5); dma_raw(6)
    dma_oh(3); dma_em(3)
    dma_raw(7)

    # ---------- constants ----------
    ones48 = const.tile([T, 1], BF16)
    nc.vector.memset(ones48[:], 1.0)
    ones96 = const.tile([96, 1], BF16)
    nc.vector.memset(ones96[:], 1.0)
    ones128 = const.tile([128, 1], BF16)
    nc.vector.memset(ones128[:], 1.0)
    finc = const.tile([1, 1], F32)
    nc.vector.memset(finc[:], FINC)
    mstore = const.tile([1, 2 * B], F32)
    nc.vector.memset(mstore[:], 1.0)

    # ---------- numerator machinery ----------
    psCGE = psn.tile([96, 192], F32, tag="psCGE")
    psCG = psCGE[:, 0:96]
    psGE = psCGE[:, 96:192]
    mm_state = [0]
    NPAIR = 256

    def emit_pairs(n):
        for _ in range(n):
            k = mm_state[0]
            if k >= NPAIR:
                return
            g, i = divmod(k, 64)
            stat = ohxt[g][:, (2 * i + 1) * T:(2 * i + 3) * T]
            mvt = ohxt[g][:, 2 * i * T:(2 * i + 2) * T]
            nc.tensor.matmul(psCG, stat, mvt, start=(k == 0),
                             stop=(k == NPAIR - 1), skip_group_check=True)
            nc.tensor.matmul(psGE, stat, emt[g][:, 2 * i * T:(2 * i + 2) * T],
                             start=(k == 0), stop=(k == NPAIR - 1),
                             skip_group_check=True)
            mm_state[0] = k + 1

    # ---------- warmup (A-fwd rows 0:48, B-fwd rows 64:112, stacked) ----------
    wf = statep.tile([112, B], BF16, tag="wstate", name="wst")
    nc.vector.memset(wf[:], 1.0)
    for j in range(W):
        ps = psp.tile([112, B], F32, tag="ps0", name=f"wps{j}")
        nc.tensor.matmul(ps[:, :], wstat112[:], wf[:, :], start=True, stop=True)
        wcs = slice(j * B, (j + 1) * B)
        if j < W - 1:
            nf = statep.tile([112, B], BF16, tag="wstate", name=f"wst{j}")
            nc.vector.tensor_mul(nf[:, :], ps[:, :], wslr[:, wcs])
            wf = nf
        else:
            nc.vector.tensor_mul(SS[0][0:T, :], ps[0:T, :], wslr[0:T, wcs])
            nc.vector.tensor_mul(SS[1][0:T, :], ps[64:112, :], wslr[64:112, wcs])

    # warmup boundary norms: mxwA (weighted by rwrow) and mxwB (always -1)
    mxw = psr.tile([1, 2 * B], F32, tag="mx")
    for sub in (0, 1):
        nc.tensor.matmul(mxw[0:1, sub * B:(sub + 1) * B], ones48[:],
                         SS[sub][0:T, :], start=True, stop=True)
    nc.scalar.activation(mstore[:], mxw[:], AF.Copy)
    # warmup-log corrections during the scan (Ln on ACT loads the table early;
    # the combines on the idle gpsimd engine)
    lnm = smallp.tile([1, 2 * B], F32, tag="lnm")
    nc.scalar.activation(lnm[:], mstore[:], AF.Ln)
    wc1 = smallp.tile([1, B], F32, tag="wc1")
    nc.gpsimd.tensor_mul(wc1[:], lnm[0:1, 0:B], rwr[:])
    wcorr = smallp.tile([1, B], F32, tag="wcorr")
    nc.gpsimd.tensor_sub(wcorr[:], wc1[:], lnm[0:1, B:2 * B])

    # ---------- fused loop: 31 steps x 2 sub-segment chains ----------
    for j in range(NF):
        for sub in (0, 1):
            c = 2 * j + sub
            ci, sl = divmod(c, CHUNK)
            if j == 0:
                stat_m = stat0t if sub == 0 else bigm
            else:
                stat_m = bigm
            ps = psp.tile([112, B], F32, tag=f"ps{sub}", name=f"ps{j}_{sub}")
            nc.tensor.matmul(ps[:, :], stat_m[:], SS[sub][:, :],
                             start=True, stop=True)
            nstate = statep.tile([112, B], BF16, tag=f"state{sub}",
                                 name=f"st{j}_{sub}")
            nc.vector.tensor_mul(nstate[:, :], ps[:, :],
                                 raws[ci][:, sl * B:(sl + 1) * B])
            SS[sub] = nstate
        if j >= 2:
            emit_pairs(9)

    emit_pairs(64)

    # ---------- combines: pz_s = sum_t (A f)*e_mid*(A^T h) ----------
    pzp = psr.tile([1, 2 * B], F32, tag="mx")
    for sub in (0, 1):
        midc = 62 + sub  # chunk 7, cols 6 and 7
        psF = psp.tile([112, B], F32, tag=f"ps{sub}", name=f"psF{sub}")
        psH = psb.tile([112, B], F32, tag="bc", name=f"psH{sub}")
        nc.tensor.matmul(psF[0:T, :], bigm[:, 0:T], SS[sub][:, :],
                         start=True, stop=True)
        nc.tensor.matmul(psH[0:T, :], bigm[:, 64:112], SS[sub][:, :],
                         start=True, stop=True)
        z1 = smallp.tile([T, B], F32, tag=f"z1_{sub}")
        nc.vector.tensor_mul(z1[:], psF[0:T, :],
                             raws[7][0:T, (midc - 56) * B:(midc - 55) * B])
        z2 = smallp.tile([T, B], BF16, tag=f"z2_{sub}")
        with nc.allow_low_precision(reason="z products; log tolerant"):
            nc.vector.tensor_mul(z2[:], z1[:], psH[0:T, :])
        nc.tensor.matmul(pzp[0:1, sub * B:(sub + 1) * B], ones48[:], z2[:],
                         start=True, stop=True)

    lzv = smallp.tile([1, 2 * B], F32, tag="lzv")
    nc.scalar.activation(lzv[:], pzp[:], AF.Ln)
    acc1 = smallp.tile([1, B], F32, tag="acc1")
    nc.vector.tensor_add(acc1[:], lzv[0:1, 0:B], lzv[0:1, B:2 * B])
    acc2 = smallp.tile([1, B], F32, tag="accA", name="acc2")
    nc.vector.tensor_add(acc2[:], acc1[:], wcorr[:])
    lzsum = smallp.tile([1, 1], F32, tag="lzsum")
    nc.vector.tensor_reduce(lzsum[:], acc2[:], axis=AX.X, op=AL.add)

    # edge dot
    cntp = psx.tile([128, 1], F32, tag="x", name="cntp")
    for g in range(4):
        nc.tensor.matmul(cntp[0:T, :], edgt[:, g * T:(g + 1) * T], ones128[:],
                         start=(g == 0), stop=(g == 3), skip_group_check=True)
    dots = smallp.tile([T, 1], BF16, tag="dots")
    with nc.allow_low_precision(reason="scalar total; tolerant"):
        nc.vector.tensor_mul(dots[:], cntp[0:T, :], edgv[:])
    edsump = psx.tile([128, 1], F32, tag="x", name="edsump")
    nc.tensor.matmul(edsump[0:1, :], dots[:], ones48[:], start=True, stop=True)
    edsum = smallp.tile([1, 1], F32, tag="edsum")
    nc.scalar.activation(edsum[:], edsump[0:1, :], AF.Copy)

    # ---------- drain remaining waves ----------
    emit_pairs(NPAIR - mm_state[0])

    # gtsum
    ct96 = smallp.tile([96, 96], F32, tag="ct96")
    nc.vector.tensor_mul(ct96[:], psCG, trN96[:])
    ctr = smallp.tile([96, 1], F32, tag="ctr")
    nc.vector.tensor_reduce(ctr[:], ct96[:], axis=AX.X, op=AL.add)
    ctrb = smallp.tile([96, 1], BF16, tag="ctrb")
    with nc.allow_low_precision(reason="scalar total; tolerant"):
        nc.vector.tensor_copy(ctrb[:], ctr[:])
    gtsump = psx.tile([128, 1], F32, tag="x", name="gtsump")
    nc.tensor.matmul(gtsump[0:1, :], ctrb[:], ones96[:], start=True, stop=True)
    gtsum = smallp.tile([1, 1], F32, tag="gtsum")
    nc.scalar.activation(gtsum[:], gtsump[0:1, :], AF.Copy)

    # gesum
    dge = smallp.tile([96, 96], F32, tag="dge")
    nc.vector.tensor_mul(dge[:], psGE, id96[:])
    dger = smallp.tile([96, 1], F32, tag="dger")
    nc.vector.tensor_reduce(dger[:], dge[:], axis=AX.X, op=AL.add)
    dgerb = smallp.tile([96, 1], BF16, tag="dgerb")
    with nc.allow_low_precision(reason="scalar total; tolerant"):
        nc.vector.tensor_copy(dgerb[:], dger[:])
    gesump = psx.tile([128, 1], F32, tag="x", name="gesump")
    nc.tensor.matmul(gesump[0:1, :], dgerb[:], ones96[:], start=True, stop=True)
    gesum = smallp.tile([1, 1], F32, tag="gesum")
    nc.scalar.activation(gesum[:], gesump[0:1, :], AF.Copy)

    # total = lzsum + FINC - gesum - gtsum - edsum
    t1 = smallp.tile([1, 1], F32, tag="t1")
    nc.vector.scalar_tensor_tensor(t1[:], lzsum[:], finc[:], gesum[:],
                                   op0=AL.add, op1=AL.subtract)
    t2 = smallp.tile([1, 1], F32, tag="t2")
    nc.vector.scalar_tensor_tensor(t2[:], t1[:], gtsum[:], edsum[:],
                                   op0=AL.subtract, op1=AL.subtract)
    nc.sync.dma_start(outd[:, :], t2[:])
    ctx.close()


def _prep_core_inputs(c, em, emexp, tags, transitions, start, end,
                      trTE, trNE):
    a0 = SEG * c
    # interleaved slab layout: col 2j = sub-A slab j, col 2j+1 = sub-B slab j
    # sub-A: fwd j -> slot j, bwd j -> slot 62-j, mid (j=31) -> slot 31
    # sub-B: fwd j -> slot 64+j, bwd j -> slot 126-j, mid (j=31) -> slot 95
    emstack = np.zeros((112, 64, B), dtype=np.float32)
    for j in range(NF):
        emstack[0:T, 2 * j] = emexp[:, a0 + j, :].T
        emstack[64:112, 2 * j] = emexp[:, a0 + 62 - j, :].T
        emstack[0:T, 2 * j + 1] = emexp[:, a0 + 64 + j, :].T
        emstack[64:112, 2 * j + 1] = emexp[:, a0 + 126 - j, :].T
    emstack[0:T, 62] = emexp[:, a0 + 31, :].T
    emstack[0:T, 63] = emexp[:, a0 + 95, :].T
    emstack = emstack.reshape(112, 64 * B).astype(bf16np)

    # warmup slabs: rows 0:48 = sub-A (crafted on core 0), rows 64:112 = sub-B
    wslv = np.zeros((112, W, B), dtype=np.float32)
    if c == 0:
        wslv[0:T, :W - 1, :] = 1.0
        wslv[0:T, W - 1, :] = np.exp(start)[:, None]
    else:
        for j in range(W):
            wslv[0:T, j, :] = emexp[:, a0 - W + j, :].T
    for j in range(W):
        wslv[64:112, j, :] = emexp[:, a0 + SUB - W + j, :].T
    wslv = wslv.reshape(112, W * B).astype(bf16np)

    binitv = np.zeros((T, 2 * B), dtype=np.float32)
    binitv[:, 0:B] = emexp[:, a0 + 63, :].T
    binitv[:, B:2 * B] = emexp[:, a0 + 127, :].T
    if c == NCORES - 1:
        binitv[:, B:2 * B] *= np.exp(end)[:, None]

    # warmup stationary: A-block = eye (core 0) / fwd trans; B-block = fwd trans
    wstatv = np.zeros((112, 112), dtype=np.float32)
    wstatv[0:T, 0:T] = np.eye(T, dtype=np.float32) if c == 0 else trTE
    wstatv[64:112, 64:112] = trTE
    # first fused step of chain A: fwd block eye on core 0 (alpha_0 has no
    # transition matmul), bwd block always the bwd transitions
    stat0v = np.zeros((112, 112), dtype=np.float32)
    stat0v[0:T, 0:T] = np.eye(T, dtype=np.float32) if c == 0 else trTE
    stat0v[64:112, 64:112] = trNE

    bigmv = np.zeros((112, 112), dtype=np.float32)
    bigmv[0:T, 0:T] = trTE
    bigmv[64:112, 64:112] = trNE

    trN96 = np.zeros((96, 96), dtype=np.float32)
    trN96[0:T, 0:T] = transitions
    trN96[T:96, T:96] = transitions

    emn = em[:, a0:a0 + SEG, :]
    empe = emn.reshape(4, 128, SEG, T).transpose(1, 0, 2, 3).reshape(128, 4 * GW)

    tg = tags[:, a0:a0 + SEG].astype(np.int32)
    iot = np.arange(T, dtype=np.int32)
    tgg = tg.reshape(4, 128, SEG).transpose(1, 0, 2)  # [128, 4, SEG]
    oh = (tgg[..., None] == iot).astype(np.float32)   # [128, 4, SEG, T]
    if c == 0:
        bndv = np.zeros((128, 4, 1, T), dtype=np.float32)
    else:
        pv = tags[:, a0 - 1].astype(np.int32).reshape(4, 128).T  # [128, 4]
        bndv = (pv[:, :, None, None] == iot[None, None, None, :]).astype(
            np.float32)
    ohxv = np.concatenate([bndv, oh], axis=2).reshape(128, 4 * OHW)

    if c == 0:
        ev = tags[:, 0].astype(np.int32).reshape(4, 128).T
        edgeohv = (ev[:, :, None] == iot[None, None, :]).astype(
            np.float32).reshape(128, 4 * T)
        edgevecv = start[:, None].astype(np.float32)
        rwrowv = np.zeros((1, B), dtype=np.float32)
    elif c == NCORES - 1:
        ev = tags[:, S - 1].astype(np.int32).reshape(4, 128).T
        edgeohv = (ev[:, :, None] == iot[None, None, :]).astype(
            np.float32).reshape(128, 4 * T)
        edgevecv = end[:, None].astype(np.float32)
        rwrowv = np.full((1, B), -1.0, dtype=np.float32)
    else:
        edgeohv = np.zeros((128, 4 * T), dtype=np.float32)
        edgevecv = np.zeros((T, 1), dtype=np.float32)
        rwrowv = np.full((1, B), -1.0, dtype=np.float32)

    return {
        "emstack": emstack,
        "wsl": wslv,
        "binit": binitv.astype(bf16np),
        "wstat": wstatv.astype(bf16np),
        "stat0": stat0v.astype(bf16np),
        "bigmd": bigmv.astype(bf16np),
        "trN96d": trN96,
        "id96d": np.eye(96, dtype=np.float32).astype(bf16np),
        "empe": empe.astype(fp8np),
        "ohx": ohxv.astype(fp8np),
        "edgeoh": edgeohv.astype(bf16np),
        "edgevec": edgevecv,
        "rwrow": rwrowv,
    }


def prep_all_inputs(emissions, tags, mask, transitions, start_transitions,
                    end_transitions):
    em = np.asarray(emissions, dtype=np.float32)
    emexp = np.exp(em - CBIAS).astype(np.float32)
    tg = np.asarray(tags)
    tr = np.asarray(transitions, dtype=np.float32)
    st = np.asarray(start_transitions, dtype=np.float32)
    en = np.asarray(end_transitions, dtype=np.float32)
    trTE = np.exp(tr.T).astype(np.float32)
    trNE = np.exp(tr).astype(np.float32)
    return [_prep_core_inputs(c, em, emexp, tg, tr, st, en, trTE, trNE)
            for c in range(NCORES)]


_NC_CACHE = {}


def get_graph():
    if "nc" not in _NC_CACHE:
        _NC_CACHE["nc"] = _build_graph()
    return _NC_CACHE["nc"]


def kernel(emissions, tags, mask, transitions, start_transitions, end_transitions,
           **kw):
    from concourse import bass_utils
    nc = get_graph()
    in_maps = prep_all_inputs(emissions, tags, mask, transitions,
                              start_transitions, end_transitions)
    res = bass_utils.run_bass_kernel_spmd(nc, in_maps, core_ids=list(range(NCORES)))
    total = sum(float(res.results[c]["out"][0, 0]) for c in range(NCORES))
    return np.float32(total / B)


if __name__ == "__main__":
    get_graph()
    print("graph built ok")
